# revision 1
# baseline (speedup 1.0000x reference)
"""GroupWiseLinear Trainium2 kernel.

out[b, c] = dot(W[0, c, :], x[b, group_of[c], :]) + bias[0, c], then a final
class-permutation gather, for two independent branches (co / cl).

Sharding: 8 cores = 2 branches x 4 class-quarters (1024 classes each, all 64
batches per core).  The ragged group segments of each core's class range are
split/padded on host into uniform 64-column "slots" so that every core runs
the SAME instruction stream (SPMD) on different data:

  - xt:  [128, S*4*64]  per-slot x^T (H-major), replicated per slot
  - wt:  [128, 4*S*64]  W^T (H-major), zero-padded to slot layout
  - bz:  [1, S*64]      bias, zero-padded to slot layout
  - out: [64, S*64]     padded per-core output (batch-major)

Device work per slot: 4 K-chunk matmuls (x stationary [128,64], W moving) that
accumulate into PSUM, plus a rank-1 ones-matmul adding the bias.  Host
"unshard" places each core's real columns into the final permuted output.
"""

import ml_dtypes
import numpy as np

import concourse.bacc as bacc
import concourse.tile as tile
from concourse import mybir
from concourse.bass_utils import run_bass_kernel_spmd

B = 64          # batch
H = 512         # hidden
NC_CLS = 4096   # classes per branch
NQ = 4          # class-quarters per branch
QCLS = NC_CLS // NQ
KC = H // 128   # contraction chunks

_cache = {}


def _build_shards(co_group_of, cl_group_of):
    """Per (branch, quarter): list of slots (group, cls_start, width<=64)."""
    shards = []
    for go in (co_group_of, cl_group_of):
        go = np.asarray(go).astype(np.int64)
        for q in range(NQ):
            c0, c1 = q * QCLS, (q + 1) * QCLS
            slots = []
            i = c0
            while i < c1:
                g = go[i]
                j = i
                while j < c1 and go[j] == g:
                    j += 1
                for s in range(i, j, 64):
                    slots.append((int(g), s, min(64, j - s)))
                i = j
            shards.append(slots)
    return shards


def _program(S, dt=mybir.dt.bfloat16):
    """Build the uniform SPMD Bass program for S slots per core."""
    nc = bacc.Bacc("TRN2", target_bir_lowering=False, debug=False, num_devices=8)
    xt_d = nc.dram_tensor("xt", [128, S * KC * 64], dt, kind="ExternalInput")
    wt_d = nc.dram_tensor("wt", [128, KC, S * 64], dt, kind="ExternalInput")
    bz_d = nc.dram_tensor("bz", [1, S * 64], dt, kind="ExternalInput")
    nhalf = ((S * 64 + 511) // 512 + 1) // 2
    o_d = nc.dram_tensor("o", [128, 512 * nhalf], mybir.dt.float32, kind="ExternalOutput")

    ntiles = (S * 64 + 511) // 512

    with tile.TileContext(nc) as tc:
        with (
            tc.tile_pool(name="xp", bufs=4 * ntiles) as xp,
            tc.tile_pool(name="wp", bufs=ntiles * KC) as wp,
            tc.tile_pool(name="cp", bufs=1) as cp,
            tc.tile_pool(name="op", bufs=ntiles) as op,
            tc.tile_pool(name="ps", bufs=min(ntiles, 8), space="PSUM") as ps,
        ):
            ones = cp.tile([1, 64], dt)
            nc.gpsimd.memset(ones[:], 1.0)
            bz = cp.tile([1, S * 64], dt)
            nc.scalar.dma_start(bz[:], bz_d[:])

            ohs = []
            for t in range(ntiles):
                s_lo = t * 8
                s_hi = min(S, s_lo + 8)
                nsl = s_hi - s_lo
                tw = nsl * 64

                xt = xp.tile([128, nsl * KC * 64], dt)
                nc.sync.dma_start(xt[:], xt_d[:, s_lo * KC * 64 : s_hi * KC * 64])
                wt = wp.tile([128, KC, tw], dt)
                nc.scalar.dma_start(wt[:], wt_d[:, :, s_lo * 64 : s_hi * 64])

                acc = ps.tile([64, 512], mybir.dt.float32)
                for sl in range(nsl):
                    for k in range(KC):
                        nc.tensor.matmul(
                            acc[0:64, sl * 64 : (sl + 1) * 64],
                            xt[:, (sl * KC + k) * 64 : (sl * KC + k + 1) * 64],
                            wt[:, k, sl * 64 : (sl + 1) * 64],
                            start=(k == 0),
                            stop=False,
                        )
                    nc.tensor.matmul(
                        acc[0:64, sl * 64 : (sl + 1) * 64],
                        ones[0:1, 0:64],
                        bz[0:1, (s_lo + sl) * 64 : (s_lo + sl + 1) * 64],
                        start=False,
                        stop=True,
                    )

                if t % 2 == 0:
                    oh = op.tile([128, 512], mybir.dt.float32)
                    ohs.append(oh)
                oh = ohs[t // 2]
                r0 = 64 * (t % 2)
                nc.vector.tensor_copy(oh[r0 : r0 + 64, 0:tw], acc[0:64, 0:tw])
                if t % 2 == 1 or t == ntiles - 1:
                    h = t // 2
                    eng = nc.sync if h % 2 == 0 else nc.scalar
                    eng.dma_start(o_d[:, h * 512 : (h + 1) * 512], oh[:])

    nc.compile()
    return nc


def _host_prep(x, W, bias, slots, S, goff):
    """Build xt/wt/bz arrays for one core."""
    nsl = len(slots)
    groups = np.array([g for g, _, _ in slots], np.int64)
    # xt: [128, S*KC*64]; col = s*(KC*64) + k*64 + b
    xg = x[:, goff + groups, :]                      # [B, nsl, H]
    xt = np.zeros((128, S * KC * 64), ml_dtypes.bfloat16)
    xt[:, : nsl * KC * 64] = (
        xg.reshape(B, nsl, KC, 128).transpose(3, 1, 2, 0).reshape(128, nsl * KC * 64)
    )
    # wt: [128, KC*S*64]; col = k*(S*64) + s*64 + j
    Wp = np.zeros((S * 64, H), ml_dtypes.bfloat16)
    bz = np.zeros((1, S * 64), ml_dtypes.bfloat16)
    for s, (g, cst, wdt) in enumerate(slots):
        Wp[s * 64 : s * 64 + wdt] = W[cst : cst + wdt]
        bz[0, s * 64 : s * 64 + wdt] = bias[cst : cst + wdt]
    wt = Wp.reshape(S * 64, KC, 128).transpose(2, 1, 0).reshape(128, KC * S * 64)
    return {"xt": xt, "wt": np.ascontiguousarray(wt).reshape(128, KC, S * 64), "bz": bz}


def kernel(x, co_W, cl_W, co_b, cl_b, co_group_of, cl_group_of, co_index,
           cl_index, group_len, _iters=1, _return_raw=False):
    x = np.asarray(x, np.float32)
    G = int(group_len)
    shards = _build_shards(co_group_of, cl_group_of)
    S = max(len(s) for s in shards)

    key = ("v5bf16", S)
    if key not in _cache:
        _cache[key] = _program(S)
    nc = _cache[key]

    Ws = (np.asarray(co_W, np.float32)[0], np.asarray(cl_W, np.float32)[0])
    bs = (np.asarray(co_b, np.float32)[0], np.asarray(cl_b, np.float32)[0])
    in_maps = []
    for k in range(8):
        bi, q = divmod(k, NQ)
        in_maps.append(_host_prep(x, Ws[bi], bs[bi], shards[k], S, bi * G))

    res = run_bass_kernel_spmd(nc, in_maps, list(range(8)))

    outs = []
    for bi, index in ((0, co_index), (1, cl_index)):
        full = np.empty((B, NC_CLS), np.float32)
        for q in range(NQ):
            slots = shards[bi * NQ + q]
            src = np.empty(QCLS, np.int64)
            for s, (g, cst, wdt) in enumerate(slots):
                src[cst - q * QCLS : cst - q * QCLS + wdt] = np.arange(
                    s * 64, s * 64 + wdt
                )
            oarr = res.results[bi * NQ + q]["o"]
            ntiles = (S * 64 + 511) // 512
            flat = np.empty((B, S * 64), np.float32)
            for t in range(ntiles):
                s_lo, s_hi = t * 8, min(S, t * 8 + 8)
                tw = (s_hi - s_lo) * 64
                r0 = 64 * (t % 2)
                flat[:, s_lo * 64 : s_lo * 64 + tw] = oarr[
                    r0 : r0 + 64, (t // 2) * 512 : (t // 2) * 512 + tw
                ]
            full[:, q * QCLS : (q + 1) * QCLS] = flat[:, src]
        outs.append(full[:, np.asarray(index).astype(np.int64)])
    return outs[0], outs[1]



# revision 3
# speedup vs baseline: 1.2697x; 1.2697x over previous
"""GroupWiseLinear Trainium2 kernel.

out[b, c] = dot(W[0, c, :], x[b, group_of[c], :]) + bias[0, c], then a final
class-permutation gather, for two independent branches (co / cl).

Sharding: 8 cores = 2 branches x 4 class-shards.  Shard boundaries are chosen
at group boundaries so no group's x is loaded by two cores.  Each core's
ragged class range is cut into "pieces" (one group each, <= 512 classes); the
piece widths of all 8 cores are rank-matched into a single static width
ENVELOPE so every core runs the same instruction stream (SPMD) on different
data:

  - xm: [128, S, 256]    per-slot x^T (one 64KB tile per piece, H-major)
  - wt: [128, 4, CW]     W^T packed to the envelope layout (pad = garbage)
  - o:  [64, CW]         bf16 output, envelope layout (pad ignored on host)

Device work per slot: 4 K-chunk matmuls (x stationary [128,64], W moving
[128, w_env]) accumulating into a PSUM bank shared by several slots; each
bank is then copied (f32->bf16, two engine-parallel halves) to SBUF and
DMA'd out.  Bias and the final class permutation are applied on host.
"""

import heapq

import ml_dtypes
import numpy as np

import concourse.bacc as bacc
import concourse.tile as tile
from concourse import mybir
from concourse.bass_utils import run_bass_kernel_spmd

B = 64          # batch
H = 512         # hidden
NCLS = 4096     # classes per branch
KC = H // 128   # contraction chunks
NQ = 4          # class-shards per branch
BANK = 512      # psum bank width (f32 cols)

_cache = {}


# ----------------------------------------------------------------- planning

def _segments(go):
    """group_of (sorted) -> list of (group, class_start, width)."""
    go = np.asarray(go).astype(np.int64)
    segs = []
    i = 0
    n = len(go)
    while i < n:
        j = i
        while j < n and go[j] == go[i]:
            j += 1
        segs.append((int(go[i]), i, j - i))
        i = j
    return segs


def _core_pieces(segs, S):
    """Split a core's segments into exactly S pieces (halve the largest),
    returning them sorted by descending width.  None if > S segments."""
    if len(segs) > S:
        return None
    h = [(-w, g, cs, w) for (g, cs, w) in segs]
    heapq.heapify(h)
    n = len(h)
    while n < S:
        _, g, cs, w = heapq.heappop(h)
        a = w // 2
        b = w - a
        if a == 0:  # cannot split further; put back and stop
            heapq.heappush(h, (-w, g, cs, w))
            break
        heapq.heappush(h, (-a, g, cs, a))
        heapq.heappush(h, (-b, g, cs + a, b))
        n += 1
    return sorted(((w, g, cs) for (_, g, cs, w) in h), reverse=True)


def _branch_cores(segs, cuts, S):
    """cuts: 3 group-index boundaries -> 4 cores' piece lists."""
    bounds = [0] + list(cuts) + [len(segs)]
    out = []
    for a, b in zip(bounds[:-1], bounds[1:]):
        if a >= b:
            return None
        p = _core_pieces(segs[a:b], S)
        if p is None:
            return None
        out.append(p)
    return out


def _envelope(cores, S):
    env = [0] * S
    for pieces in cores:
        for i, (w, _, _) in enumerate(pieces):
            if w > env[i]:
                env[i] = w
    return env


def _cost(cores_all, S):
    env = _envelope(cores_all, S)
    return 64 * S + sum(env), env


def _plan_cuts(segs_co, segs_cl):
    """Choose S and per-branch cuts minimizing 64*S + sum(envelope)."""
    def balanced(segs):
        widths = np.array([w for (_, _, w) in segs])
        csum = np.cumsum(widths)
        cuts = []
        for i in range(1, NQ):
            cuts.append(int(np.argmin(np.abs(csum - i * csum[-1] / NQ))) + 1)
        return tuple(cuts)

    smin = 1
    for segs in (segs_co, segs_cl):
        c = balanced(segs)
        b = [0] + list(c) + [len(segs)]
        smin = max(smin, max(b[i + 1] - b[i] for i in range(NQ)))

    best = None
    for S in range(max(2, smin - 2), smin + 5):
        cuts = {}
        cores = {}
        ok = True
        for name, segs in (("co", segs_co), ("cl", segs_cl)):
            c = balanced(segs)
            cs = _branch_cores(segs, c, S)
            if cs is None:
                # widen: fall back to equal group counts
                n = len(segs)
                c = (n // 4, n // 2, 3 * n // 4)
                cs = _branch_cores(segs, c, S)
            if cs is None:
                ok = False
                break
            cuts[name] = c
            cores[name] = cs
        if not ok:
            continue
        for _ in range(3):
            improved = False
            for name, segs in (("co", segs_co), ("cl", segs_cl)):
                other = cores["cl" if name == "co" else "co"]
                c0, c1, c2 = cuts[name]
                bloc = None
                for d0 in range(-3, 4):
                    for d1 in range(-3, 4):
                        for d2 in range(-3, 4):
                            cc = (c0 + d0, c1 + d1, c2 + d2)
                            if not (0 < cc[0] < cc[1] < cc[2] < len(segs)):
                                continue
                            cs = _branch_cores(segs, cc, S)
                            if cs is None:
                                continue
                            cost, _ = _cost(cs + other, S)
                            if bloc is None or cost < bloc[0]:
                                bloc = (cost, cc, cs)
                if bloc is not None and bloc[1] != cuts[name]:
                    cuts[name] = bloc[1]
                    cores[name] = bloc[2]
                    improved = True
            if not improved:
                break
        cost, env = _cost(cores["co"] + cores["cl"], S)
        if best is None or cost < best[0]:
            best = (cost, S, cores["co"] + cores["cl"], env)
    _, S, cores8, env = best
    return S, cores8, env


def _pack_banks(env):
    """First-fit-decreasing pack envelope widths into <=512 psum banks, then
    split the last bank so the tail bank is small.  Returns (slot_order,
    banks) where banks = list of lists of slot-ranks."""
    banks = []
    fill = []
    for i, w in enumerate(env):
        placed = False
        for b in range(len(banks)):
            if fill[b] + w <= BANK:
                banks[b].append(i)
                fill[b] += w
                placed = True
                break
        if not placed:
            banks.append([i])
            fill.append(w)
    # make the tail bank small: peel smallest slots of the last bank
    if fill[-1] > 192 and len(banks[-1]) > 1:
        tailw = 0
        tail = []
        rest = list(banks[-1])
        for i in sorted(rest, key=lambda i: env[i]):
            if tailw + env[i] > 128:
                break
            tail.append(i)
            tailw += env[i]
        if tail:
            rest = [i for i in rest if i not in tail]
            banks[-1] = rest
            banks.append(sorted(tail))
    return banks


def _plan(co_go, cl_go):
    segs_co = _segments(co_go)
    segs_cl = _segments(cl_go)
    S, cores8, env = _plan_cuts(segs_co, segs_cl)
    banks = _pack_banks(env)
    # final slot order: bank-major
    order = [i for bk in banks for i in bk]
    rank_to_slot = {r: s for s, r in enumerate(order)}
    widths = [env[r] for r in order]               # per final slot
    offs = np.concatenate([[0], np.cumsum(widths)]).astype(np.int64)
    CW = int(offs[-1])
    bank_meta = []                                  # (slot_lo, nslots, c_lo, c_hi)
    s = 0
    for bk in banks:
        bank_meta.append((s, len(bk), int(offs[s]), int(offs[s + len(bk)])))
        s += len(bk)
    # per-core slot fill: list over cores of list of (slot, w_real, g, cstart)
    core_fill = []
    for pieces in cores8:
        fill = []
        for r, (w, g, cs) in enumerate(pieces):
            fill.append((rank_to_slot[r], w, g, cs))
        core_fill.append(fill)
    return {
        "S": len(order), "env": tuple(widths), "offs": offs, "CW": CW,
        "banks": bank_meta, "core_fill": core_fill,
    }


# ----------------------------------------------------------------- program

def _program(env, banks, dt=mybir.dt.bfloat16):
    S = len(env)
    offs = np.concatenate([[0], np.cumsum(env)]).astype(np.int64)
    CW = int(offs[-1])
    nc = bacc.Bacc("TRN2", target_bir_lowering=False, debug=False, num_devices=8)
    xm_d = nc.dram_tensor("xm", [128, S, KC * 64], dt, kind="ExternalInput")
    wt_d = nc.dram_tensor("wt", [128, KC, CW], dt, kind="ExternalInput")
    o_d = nc.dram_tensor("o", [64, CW], dt, kind="ExternalOutput")

    nbk = len(banks)
    with tile.TileContext(nc) as tc:
        with (
            tc.tile_pool(name="xp", bufs=1) as xp,
            tc.tile_pool(name="wp", bufs=1) as wp,
            tc.tile_pool(name="op", bufs=1) as op,
            tc.tile_pool(name="ps", bufs=1, space="PSUM") as ps,
        ):
            xts = []
            wts = []
            for bi, (s_lo, nsl, c_lo, c_hi) in enumerate(banks):
                xt = xp.tile([128, nsl, KC * 64], dt, name=f"xt{bi}")
                nc.sync.dma_start(xt[:], xm_d[:, s_lo : s_lo + nsl, :])
                wt = wp.tile([128, KC, c_hi - c_lo], dt, name=f"wt{bi}")
                nc.scalar.dma_start(wt[:], wt_d[:, :, c_lo:c_hi])
                xts.append(xt)
                wts.append(wt)

            for bi, (s_lo, nsl, c_lo, c_hi) in enumerate(banks):
                used = c_hi - c_lo
                xt, wt = xts[bi], wts[bi]
                acc = ps.tile([64, BANK], mybir.dt.float32, name=f"acc{bi}")
                for j in range(nsl):
                    s = s_lo + j
                    f0 = int(offs[s] - c_lo)
                    w = env[s]
                    for k in range(KC):
                        nc.tensor.matmul(
                            acc[0:64, f0 : f0 + w],
                            xt[:, j, k * 64 : (k + 1) * 64],
                            wt[:, k, f0 : f0 + w],
                            start=(k == 0),
                            stop=(k == KC - 1),
                        )
                ob = op.tile([64, used], dt, name=f"ob{bi}")
                half = (used // 2) & ~1
                if half >= 64:
                    nc.vector.tensor_copy(ob[:, 0:half], acc[0:64, 0:half])
                    nc.scalar.copy(ob[:, half:used], acc[0:64, half:used])
                else:
                    nc.vector.tensor_copy(ob[:], acc[0:64, 0:used])
                eng = nc.sync if bi % 2 == 0 else nc.scalar
                eng.dma_start(o_d[:, c_lo:c_hi], ob[:])

    nc.compile()
    return nc


# ----------------------------------------------------------------- host glue

def _host_prep(x, W, fill, S, CW, offs, goff):
    """Build xm/wt for one core.  fill: list of (slot, w_real, g, cstart)."""
    bf = ml_dtypes.bfloat16
    xm = np.zeros((128, S, KC * 64), bf)
    wt = np.zeros((128, KC, CW), bf)
    for slot, w, g, cs in fill:
        # x[b, g, h] -> [128 part(h%128), k, b]
        xg = x[:, goff + g, :].reshape(B, KC, 128).transpose(2, 1, 0)
        xm[:, slot, :] = xg.reshape(128, KC * 64).astype(bf)
        # W[c, h] -> [128, k, w]
        wg = W[cs : cs + w].reshape(w, KC, 128).transpose(2, 1, 0)
        wt[:, :, int(offs[slot]) : int(offs[slot]) + w] = wg.astype(bf)
    return {"xm": xm, "wt": wt}


def kernel(x, co_W, cl_W, co_b, cl_b, co_group_of, cl_group_of, co_index,
           cl_index, group_len):
    x = np.asarray(x, np.float32)
    G = int(group_len)
    plan = _plan(co_group_of, cl_group_of)
    S, env, offs, CW = plan["S"], plan["env"], plan["offs"], plan["CW"]

    key = ("v6env", env, tuple(b[:2] for b in plan["banks"]))
    if key not in _cache:
        _cache[key] = _program(list(env), plan["banks"])
    nc = _cache[key]

    Ws = (np.asarray(co_W, np.float32)[0], np.asarray(cl_W, np.float32)[0])
    in_maps = []
    for c in range(8):
        bi = c // NQ
        in_maps.append(_host_prep(x, Ws[bi], plan["core_fill"][c], S, CW, offs, bi * G))

    res = run_bass_kernel_spmd(nc, in_maps, list(range(8)))

    outs = []
    for bi, bias, index in ((0, co_b, co_index), (1, cl_b, cl_index)):
        full = np.empty((B, NCLS), np.float32)
        for q in range(NQ):
            o = np.asarray(res.results[bi * NQ + q]["o"]).astype(np.float32)
            for slot, w, g, cs in plan["core_fill"][bi * NQ + q]:
                f0 = int(offs[slot])
                full[:, cs : cs + w] = o[:, f0 : f0 + w]
        full += np.asarray(bias, np.float32)
        outs.append(full[:, np.asarray(index).astype(np.int64)])
    return outs[0], outs[1]


# revision 8
# speedup vs baseline: 1.2739x; 1.0034x over previous
"""GroupWiseLinear Trainium2 kernel.

out[b, c] = dot(W[0, c, :], x[b, group_of[c], :]) + bias[0, c], then a final
class-permutation gather, for two independent branches (co / cl).

Sharding: 8 cores = 2 branches x 4 class-shards.  Shard boundaries are chosen
at group boundaries so no group's x is loaded by two cores.  Each core's
ragged class range is cut into "pieces" (one group each, <= 512 classes); the
piece widths of all 8 cores are rank-matched into a single static width
ENVELOPE so every core runs the same instruction stream (SPMD) on different
data:

  - xm: [128, S, 256]    per-slot x^T (one 64KB tile per piece, H-major)
  - wt: [128, 4, CW]     W^T packed to the envelope layout (pad = garbage)
  - o:  [64, CW]         bf16 output, envelope layout (pad ignored on host)

Device work per slot: 4 K-chunk matmuls (x stationary [128,64], W moving
[128, w_env]) accumulating into a PSUM bank shared by several slots; each
bank is then copied (f32->bf16, two engine-parallel halves) to SBUF and
DMA'd out.  Bias and the final class permutation are applied on host.
"""

import heapq

import ml_dtypes
import numpy as np

import concourse.bacc as bacc
import concourse.tile as tile
from concourse import mybir
from concourse.bass_utils import run_bass_kernel_spmd

B = 64          # batch
H = 512         # hidden
NCLS = 4096     # classes per branch
KC = H // 128   # contraction chunks
NQ = 4          # class-shards per branch
BANK = 512      # psum bank width (f32 cols)

_cache = {}


# ----------------------------------------------------------------- planning

def _segments(go):
    """group_of (sorted) -> list of (group, class_start, width)."""
    go = np.asarray(go).astype(np.int64)
    segs = []
    i = 0
    n = len(go)
    while i < n:
        j = i
        while j < n and go[j] == go[i]:
            j += 1
        segs.append((int(go[i]), i, j - i))
        i = j
    return segs


def _core_pieces(segs, S):
    """Split a core's segments into exactly S pieces (halve the largest),
    returning them sorted by descending width.  None if > S segments."""
    if len(segs) > S:
        return None
    h = [(-w, g, cs, w) for (g, cs, w) in segs]
    heapq.heapify(h)
    n = len(h)
    while n < S:
        _, g, cs, w = heapq.heappop(h)
        a = w // 2
        b = w - a
        if a == 0:  # cannot split further; put back and stop
            heapq.heappush(h, (-w, g, cs, w))
            break
        heapq.heappush(h, (-a, g, cs, a))
        heapq.heappush(h, (-b, g, cs + a, b))
        n += 1
    return sorted(((w, g, cs) for (_, g, cs, w) in h), reverse=True)


def _branch_cores(segs, cuts, S):
    """cuts: 3 group-index boundaries -> 4 cores' piece lists."""
    bounds = [0] + list(cuts) + [len(segs)]
    out = []
    for a, b in zip(bounds[:-1], bounds[1:]):
        if a >= b:
            return None
        p = _core_pieces(segs[a:b], S)
        if p is None:
            return None
        out.append(p)
    return out


def _envelope(cores, S):
    env = [0] * S
    for pieces in cores:
        for i, (w, _, _) in enumerate(pieces):
            if w > env[i]:
                env[i] = w
    return env


def _cost(cores_all, S):
    env = _envelope(cores_all, S)
    return 64 * S + sum(env), env


def _plan_cuts(segs_co, segs_cl):
    """Choose S and per-branch cuts minimizing 64*S + sum(envelope)."""
    def balanced(segs):
        widths = np.array([w for (_, _, w) in segs])
        csum = np.cumsum(widths)
        cuts = []
        for i in range(1, NQ):
            cuts.append(int(np.argmin(np.abs(csum - i * csum[-1] / NQ))) + 1)
        return tuple(cuts)

    # equal-group-count cuts are always feasible at S = ceil(ngroups / NQ)
    smin = max(-(-len(segs) // NQ) for segs in (segs_co, segs_cl))

    best = None
    for S in range(smin, smin + 7):
        cuts = {}
        cores = {}
        ok = True
        for name, segs in (("co", segs_co), ("cl", segs_cl)):
            c = balanced(segs)
            cs = _branch_cores(segs, c, S)
            if cs is None:
                # widen: fall back to equal group counts
                n = len(segs)
                c = (n // 4, n // 2, 3 * n // 4)
                cs = _branch_cores(segs, c, S)
            if cs is None:
                ok = False
                break
            cuts[name] = c
            cores[name] = cs
        if not ok:
            continue
        for _ in range(3):
            improved = False
            for name, segs in (("co", segs_co), ("cl", segs_cl)):
                other = cores["cl" if name == "co" else "co"]
                c0, c1, c2 = cuts[name]
                bloc = None
                for d0 in range(-3, 4):
                    for d1 in range(-3, 4):
                        for d2 in range(-3, 4):
                            cc = (c0 + d0, c1 + d1, c2 + d2)
                            if not (0 < cc[0] < cc[1] < cc[2] < len(segs)):
                                continue
                            cs = _branch_cores(segs, cc, S)
                            if cs is None:
                                continue
                            cost, _ = _cost(cs + other, S)
                            if bloc is None or cost < bloc[0]:
                                bloc = (cost, cc, cs)
                if bloc is not None and bloc[1] != cuts[name]:
                    cuts[name] = bloc[1]
                    cores[name] = bloc[2]
                    improved = True
            if not improved:
                break
        cost, env = _cost(cores["co"] + cores["cl"], S)
        if best is None or cost < best[0]:
            best = (cost, S, cores["co"] + cores["cl"], env)
    _, S, cores8, env = best
    return S, cores8, env


def _pack_banks(env):
    """First-fit-decreasing pack envelope widths into <=512 psum banks, then
    split the last bank so the tail bank is small.  Returns (slot_order,
    banks) where banks = list of lists of slot-ranks."""
    banks = []
    fill = []
    for i, w in enumerate(env):
        placed = False
        for b in range(len(banks)):
            if fill[b] + w <= BANK:
                banks[b].append(i)
                fill[b] += w
                placed = True
                break
        if not placed:
            banks.append([i])
            fill.append(w)
    # tail bank = single smallest slot, so the last DMA->matmul->copy->DMA
    # chain after the final input transfer is as short as possible
    smallest = min(range(len(env)), key=lambda i: env[i])
    for b in range(len(banks)):
        if smallest in banks[b]:
            if len(banks[b]) == 1 and b == len(banks) - 1:
                break
            banks[b] = [i for i in banks[b] if i != smallest]
            banks = [bk for bk in banks if bk]
            banks.append([smallest])
            break
    return banks


def _plan(co_go, cl_go):
    segs_co = _segments(co_go)
    segs_cl = _segments(cl_go)
    S, cores8, env = _plan_cuts(segs_co, segs_cl)
    banks = _pack_banks(env)
    # final slot order: bank-major
    order = [i for bk in banks for i in bk]
    rank_to_slot = {r: s for s, r in enumerate(order)}
    widths = [env[r] for r in order]               # per final slot
    offs = np.concatenate([[0], np.cumsum(widths)]).astype(np.int64)
    CW = int(offs[-1])
    bank_meta = []                                  # (slot_lo, nslots, c_lo, c_hi)
    s = 0
    for bk in banks:
        bank_meta.append((s, len(bk), int(offs[s]), int(offs[s + len(bk)])))
        s += len(bk)
    # per-core slot fill: list over cores of list of (slot, w_real, g, cstart)
    core_fill = []
    for pieces in cores8:
        fill = []
        for r, (w, g, cs) in enumerate(pieces):
            fill.append((rank_to_slot[r], w, g, cs))
        core_fill.append(fill)
    return {
        "S": len(order), "env": tuple(widths), "offs": offs, "CW": CW,
        "banks": bank_meta, "core_fill": core_fill,
    }


# ----------------------------------------------------------------- program

def _program(env, banks, dt=mybir.dt.bfloat16):
    S = len(env)
    offs = np.concatenate([[0], np.cumsum(env)]).astype(np.int64)
    CW = int(offs[-1])
    nc = bacc.Bacc("TRN2", target_bir_lowering=False, debug=False, num_devices=8)
    xm_d = nc.dram_tensor("xm", [128, S, KC * 64], dt, kind="ExternalInput")
    # wt is bank-major flat: bank b occupies cols [KC*c_lo, KC*c_hi) with
    # inner layout [KC, wb] -- keeps every DMA's contiguous run >= 512B
    wt_d = nc.dram_tensor("wt", [128, KC * CW], dt, kind="ExternalInput")
    o_d = nc.dram_tensor("o", [64, CW], dt, kind="ExternalOutput")

    nbk = len(banks)
    with tile.TileContext(nc) as tc:
        with (
            tc.tile_pool(name="xp", bufs=1) as xp,
            tc.tile_pool(name="wp", bufs=1) as wp,
            tc.tile_pool(name="op", bufs=1) as op,
            tc.tile_pool(name="ps", bufs=1, space="PSUM") as ps,
        ):
            xts = []
            wts = []
            for bi, (s_lo, nsl, c_lo, c_hi) in enumerate(banks):
                wb = c_hi - c_lo
                xt = xp.tile([128, nsl, KC * 64], dt, name=f"xt{bi}")
                nc.sync.dma_start(xt[:], xm_d[:, s_lo : s_lo + nsl, :])
                wt = wp.tile([128, KC * wb], dt, name=f"wt{bi}")
                nc.scalar.dma_start(wt[:], wt_d[:, KC * c_lo : KC * c_hi])
                xts.append(xt)
                wts.append(wt)

            for bi, (s_lo, nsl, c_lo, c_hi) in enumerate(banks):
                used = c_hi - c_lo
                xt, wt = xts[bi], wts[bi]
                acc = ps.tile([64, BANK], mybir.dt.float32, name=f"acc{bi}")
                for j in range(nsl):
                    s = s_lo + j
                    f0 = int(offs[s] - c_lo)
                    w = env[s]
                    for k in range(KC):
                        nc.tensor.matmul(
                            acc[0:64, f0 : f0 + w],
                            xt[:, j, k * 64 : (k + 1) * 64],
                            wt[:, k * used + f0 : k * used + f0 + w],
                            start=(k == 0),
                            stop=(k == KC - 1),
                        )
                ob = op.tile([64, used], dt, name=f"ob{bi}")
                half = (used // 2) & ~1
                if half >= 128:
                    nc.vector.tensor_copy(ob[:, 0:half], acc[0:64, 0:half])
                    nc.scalar.copy(ob[:, half:used], acc[0:64, half:used])
                else:
                    nc.vector.tensor_copy(ob[:], acc[0:64, 0:used])
                # keep the tail bank's output on the fast HWDGE path (sync);
                # middle banks go out via gpsimd so no SEQ blocks another
                eng = (nc.sync, nc.gpsimd, nc.scalar)[bi % 3] if bi < nbk - 1 else nc.sync
                eng.dma_start(o_d[:, c_lo:c_hi], ob[:])

    nc.compile()
    return nc


# ----------------------------------------------------------------- host glue

def _host_prep(x, W, fill, plan, goff):
    """Build xm/wt for one core.  fill: list of (slot, w_real, g, cstart)."""
    bf = ml_dtypes.bfloat16
    S, offs, CW = plan["S"], plan["offs"], plan["CW"]
    slot_bank = {}
    for s_lo, nsl, c_lo, c_hi in plan["banks"]:
        for s in range(s_lo, s_lo + nsl):
            slot_bank[s] = (c_lo, c_hi)
    xm = np.zeros((128, S, KC * 64), bf)
    wt = np.zeros((128, KC * CW), bf)
    for slot, w, g, cs in fill:
        # x[b, g, h] -> [128 part(h%128), k, b]
        xg = x[:, goff + g, :].reshape(B, KC, 128).transpose(2, 1, 0)
        xm[:, slot, :] = xg.reshape(128, KC * 64).astype(bf)
        # W[c, h] -> [128, k, w] into the bank-major flat layout
        wg = W[cs : cs + w].reshape(w, KC, 128).transpose(2, 1, 0).astype(bf)
        c_lo, c_hi = slot_bank[slot]
        used = c_hi - c_lo
        f0 = int(offs[slot]) - c_lo
        for k in range(KC):
            wt[:, KC * c_lo + k * used + f0 : KC * c_lo + k * used + f0 + w] = wg[:, k, :]
    return {"xm": xm, "wt": wt}


def kernel(x, co_W, cl_W, co_b, cl_b, co_group_of, cl_group_of, co_index,
           cl_index, group_len):
    x = np.asarray(x, np.float32)
    G = int(group_len)
    plan = _plan(co_group_of, cl_group_of)
    S, env, offs, CW = plan["S"], plan["env"], plan["offs"], plan["CW"]

    key = ("v6env", env, tuple(b[:2] for b in plan["banks"]))
    if key not in _cache:
        _cache[key] = _program(list(env), plan["banks"])
    nc = _cache[key]

    Ws = (np.asarray(co_W, np.float32)[0], np.asarray(cl_W, np.float32)[0])
    in_maps = []
    for c in range(8):
        bi = c // NQ
        in_maps.append(_host_prep(x, Ws[bi], plan["core_fill"][c], plan, bi * G))

    res = run_bass_kernel_spmd(nc, in_maps, list(range(8)))

    outs = []
    for bi, bias, index in ((0, co_b, co_index), (1, cl_b, cl_index)):
        full = np.empty((B, NCLS), np.float32)
        for q in range(NQ):
            o = np.asarray(res.results[bi * NQ + q]["o"]).astype(np.float32)
            for slot, w, g, cs in plan["core_fill"][bi * NQ + q]:
                f0 = int(offs[slot])
                full[:, cs : cs + w] = o[:, f0 : f0 + w]
        full += np.asarray(bias, np.float32)
        outs.append(full[:, np.asarray(index).astype(np.int64)])
    return outs[0], outs[1]
